# revision 12
# baseline (speedup 1.0000x reference)
"""Local causal self-attention (window=128) with RoPE — Trainium2 Bass kernel.

Sharding: 8 cores = batch(2) x 4 sequence chunks of 512 tokens.
Each core processes its 512 tokens plus a 128-token halo of prior keys
(zeros + full mask at batch starts), so no cross-core communication is
needed; the full output is a pure concatenation of per-core outputs.

Per-core pipeline (all in fp32r matmuls on the PE array):
  x[640,1024] --PE transpose--> xT[1024,640]
  qkT[2048,640] = Wqk^T xT  (+bias, RoPE via stream_shuffle; q side
                             pre-scaled by 1/sqrt(dh) in the tables)
  V[640,1024+ones] = xT^T Wv (+bias), one extra ones-column per head
  per head, per k-block: scoresT[k,q] = kT^T qT ; exp on ACT;
    multiplicative band mask; PV accumulates attn_outT[65,512]
    where row 64 = softmax denominator (from the ones column)
  per-head normalization: reciprocal row -> K=1 ones matmul broadcast
  out[512,1024] = attn_outT^T Wproj + bias
"""
import numpy as np

import concourse.bacc as bacc
import concourse.tile as tile
from concourse import mybir
from concourse.bass_utils import run_bass_kernel_spmd

F32 = mybir.dt.float32
F32R = mybir.dt.float32r

B, T, C = 2, 2048, 1024
H, DH = 16, 64
WIN = 128
ROPE_BASE = 10000.0
N_CORES = 8
CHUNK = 512          # own tokens per core
HALO = 128
L = HALO + CHUNK     # 640 local tokens
KT = C // 128        # 8 contraction tiles
NM_QK = (2 * C) // 128  # 16 m-tiles for q+k features

SWAP_MASK = []
for _i in range(16):
    SWAP_MASK += [2 * _i + 1, 2 * _i]

ExpF = mybir.ActivationFunctionType.Exp


def build_nc():
    nc = bacc.Bacc(None)

    # ---------------- DRAM I/O ----------------
    x_d = nc.dram_tensor("x_chunk", [L, C], F32R, kind="ExternalInput")
    wqkv_d = nc.dram_tensor("wqkv", [C, 3 * C], F32R, kind="ExternalInput")
    wproj_d = nc.dram_tensor("wproj", [C, C], F32R, kind="ExternalInput")
    ident_d = nc.dram_tensor("ident", [128, 128], F32R, kind="ExternalInput")
    cosq_d = nc.dram_tensor("cosq", [128, L], F32R, kind="ExternalInput")
    sinq_d = nc.dram_tensor("sinq", [128, L], F32R, kind="ExternalInput")
    cosk_d = nc.dram_tensor("cosk", [128, L], F32R, kind="ExternalInput")
    sink_d = nc.dram_tensor("sink", [128, L], F32R, kind="ExternalInput")
    m_first_d = nc.dram_tensor("mask_first", [128, 128], F32, kind="ExternalInput")
    m_mid_d = nc.dram_tensor("mask_mid", [128, 256], F32, kind="ExternalInput")
    m_last_d = nc.dram_tensor("mask_last", [128, 128], F32, kind="ExternalInput")
    bqk_d = nc.dram_tensor("bqk_cols", [128, NM_QK], F32, kind="ExternalInput")
    bv_d = nc.dram_tensor("bv_rep", [128, C], F32, kind="ExternalInput")
    bp_d = nc.dram_tensor("bp_rep", [128, C], F32, kind="ExternalInput")
    ones_col_d = nc.dram_tensor("ones_col", [1, DH], F32R, kind="ExternalInput")
    ones_v_d = nc.dram_tensor("ones_v", [128, H], F32R, kind="ExternalInput")

    out_d = nc.dram_tensor("out_chunk", [CHUNK, C], F32, kind="ExternalOutput")

    with tile.TileContext(nc) as tc:
        with (
            tc.tile_pool(name="consts", bufs=1) as cpool,
            tc.tile_pool(name="xt", bufs=1) as xtpool,
            tc.tile_pool(name="qk", bufs=1) as qkpool,
            tc.tile_pool(name="vsb", bufs=1) as vpool,
            tc.tile_pool(name="attn", bufs=1) as apool,
            tc.tile_pool(name="wp", bufs=1) as wppool,
        ):
            # ---------------- constants ----------------
            ident = cpool.tile([128, 128], F32R, tag="ident")
            nc.sync.dma_start(ident[:], ident_d[:])
            cosq = cpool.tile([128, L], F32R, tag="cosq")
            sinq = cpool.tile([128, L], F32R, tag="sinq")
            cosk = cpool.tile([128, L], F32R, tag="cosk")
            sink = cpool.tile([128, L], F32R, tag="sink")
            nc.sync.dma_start(cosq[:], cosq_d[:])
            nc.sync.dma_start(sinq[:], sinq_d[:])
            nc.sync.dma_start(cosk[:], cosk_d[:])
            nc.sync.dma_start(sink[:], sink_d[:])
            m_first = cpool.tile([128, 128], F32, tag="m_first")
            m_mid = cpool.tile([128, 256], F32, tag="m_mid")
            m_last = cpool.tile([128, 128], F32, tag="m_last")
            nc.sync.dma_start(m_first[:], m_first_d[:])
            nc.sync.dma_start(m_mid[:], m_mid_d[:])
            nc.sync.dma_start(m_last[:], m_last_d[:])
            bqk = cpool.tile([128, NM_QK], F32, tag="bqk")
            bv = cpool.tile([128, C], F32, tag="bv")
            bp = cpool.tile([128, C], F32, tag="bp")
            nc.sync.dma_start(bqk[:], bqk_d[:])
            nc.sync.dma_start(bv[:], bv_d[:])
            nc.sync.dma_start(bp[:], bp_d[:])
            ones_col = cpool.tile([1, DH], F32R, tag="ones_col")
            nc.sync.dma_start(ones_col[:], ones_col_d[:])

            # ---------------- phase 1: xT ----------------
            xT = [xtpool.tile([128, L], F32R, tag=f"xT{k}", name=f"xT{k}") for k in range(KT)]
            with (
                tc.tile_pool(name="xin", bufs=3) as xinpool,
                tc.tile_pool(name="ps_xt", bufs=2, space="PSUM") as ps_xt,
            ):
                for m5 in range(5):
                    x_sb = xinpool.tile([128, C], F32R, tag="x_sb")
                    nc.sync.dma_start(x_sb[:], x_d[m5 * 128:(m5 + 1) * 128, :])
                    for k in range(KT):
                        pt = ps_xt.tile([128, 128], F32R, tag="pt")
                        nc.tensor.transpose(
                            pt[:], x_sb[:, k * 128:(k + 1) * 128], ident[:]
                        )
                        nc.any.tensor_copy(
                            xT[k][:, m5 * 128:(m5 + 1) * 128], pt[:]
                        )

                # ---------------- phase 2: qkT + rope ----------------
                qkt = [qkpool.tile([128, L], F32R, tag=f"qk{m}", name=f"qk{m}") for m in range(NM_QK)]
                with (
                    tc.tile_pool(name="wqk", bufs=3) as wqkpool,
                    tc.tile_pool(name="ropetmp", bufs=4) as rtpool,
                    tc.tile_pool(name="ps_qk", bufs=2, space="PSUM") as ps_qk,
                ):
                    for m in range(NM_QK):
                        wqk = wqkpool.tile([128, KT, 128], F32R, tag="wqk")
                        nc.sync.dma_start(
                            wqk[:],
                            wqkv_d[:, m * 128:(m + 1) * 128].rearrange(
                                "(k p) f -> p k f", p=128
                            ),
                        )
                        psA = ps_qk.tile([128, 320], F32, tag="psA")
                        psB = ps_qk.tile([128, 320], F32, tag="psB")
                        for k in range(KT):
                            nc.tensor.matmul(psA[:], wqk[:, k, :], xT[k][:, 0:320],
                                             start=(k == 0), stop=(k == KT - 1))
                        for k in range(KT):
                            nc.tensor.matmul(psB[:], wqk[:, k, :], xT[k][:, 320:L],
                                             start=(k == 0), stop=(k == KT - 1))
                        qm = qkt[m]
                        nc.vector.tensor_scalar_add(qm[:, 0:320], psA[:], bqk[:, m:m + 1])
                        nc.vector.tensor_scalar_add(qm[:, 320:L], psB[:], bqk[:, m:m + 1])
                        # rope
                        ctab = cosq if m < 8 else cosk
                        stab = sinq if m < 8 else sink
                        t1 = rtpool.tile([128, L], F32R, tag="t1")
                        t2 = rtpool.tile([128, L], F32R, tag="t2")
                        nc.vector.stream_shuffle(t1[:].bitcast(F32), qm[:].bitcast(F32), SWAP_MASK)
                        nc.vector.tensor_mul(t1[:], t1[:], stab[:])
                        nc.vector.tensor_mul(t2[:], qm[:], ctab[:])
                        nc.vector.tensor_add(qm[:], t1[:], t2[:])

                # ---------------- phase 3: V ----------------
                v_sb = [vpool.tile([128, H * (DH + 1)], F32R, tag=f"v{m5}",
                                    name=f"v{m5}") for m5 in range(5)]
                with (
                    tc.tile_pool(name="wv", bufs=1) as wvpool,
                    tc.tile_pool(name="ps_v", bufs=3, space="PSUM") as ps_v,
                ):
                    wv = []
                    for k in range(KT):
                        wvk = wvpool.tile([128, C], F32R, tag=f"wv{k}")
                        nc.sync.dma_start(
                            wvk[:], wqkv_d[k * 128:(k + 1) * 128, 2 * C:3 * C]
                        )
                        wv.append(wvk)
                    for m5 in range(5):
                        # ones columns (col 64 of each head's 65-wide stripe)
                        vr = v_sb[m5][:].rearrange("p (h e) -> p h e", e=DH + 1)
                        nc.sync.dma_start(
                            vr[:, :, DH:DH + 1],
                            ones_v_d[:].rearrange("p (h e) -> p h e", e=1),
                        )
                        for nh in range(2):
                            pv = ps_v.tile([128, 512], F32, tag="pv")
                            for k in range(KT):
                                nc.tensor.matmul(
                                    pv[:],
                                    xT[k][:, m5 * 128:(m5 + 1) * 128],
                                    wv[k][:, nh * 512:(nh + 1) * 512],
                                    start=(k == 0), stop=(k == KT - 1),
                                )
                            nc.vector.tensor_add(
                                vr[:, nh * 8:(nh + 1) * 8, 0:DH],
                                pv[:].rearrange("p (h e) -> p h e", e=DH),
                                bv[:, nh * 512:(nh + 1) * 512].rearrange(
                                    "p (h e) -> p h e", e=DH
                                ),
                            )

            # preload Wproj (overlaps attention)
            wp = []
            for k in range(KT):
                wpk = wppool.tile([128, C], F32R, tag=f"wp{k}")
                nc.sync.dma_start(wpk[:], wproj_d[k * 128:(k + 1) * 128, :])
                wp.append(wpk)

            # ---------------- phase 4: attention ----------------
            attn = [apool.tile([128, CHUNK], F32R, tag=f"at{j}", name=f"at{j}") for j in range(KT)]
            with (
                tc.tile_pool(name="expp", bufs=6) as epool,
                tc.tile_pool(name="rcp", bufs=4) as rpool,
                tc.tile_pool(name="ps_sc", bufs=3, space="PSUM") as ps_sc,
                tc.tile_pool(name="ps_pv", bufs=2, space="PSUM") as ps_pv,
                tc.tile_pool(name="ps_rb", bufs=2, space="PSUM") as ps_rb,
            ):
                for h in range(H):
                    mt = h // 2          # m-tile pair index within q (and k) halves
                    po = (h % 2) * DH    # partition offset
                    qTh = qkt[mt]
                    kTh = qkt[8 + mt]
                    pvp = ps_pv.tile([DH + 1, CHUNK], F32, tag="pvp")
                    for kb in range(5):
                        q_lo = max(128 * kb, 128)
                        q_hi = min(128 * kb + 256, L)
                        nq = q_hi - q_lo
                        sc = ps_sc.tile([128, 256], F32, tag="sc")
                        nc.tensor.matmul(
                            sc[:, 0:nq],
                            kTh[po:po + DH, kb * 128:(kb + 1) * 128],
                            qTh[po:po + DH, q_lo:q_hi],
                            start=True, stop=True,
                        )
                        ex = epool.tile([128, 256], F32, tag="ex")
                        nc.scalar.activation(ex[:, 0:nq], sc[:, 0:nq], ExpF)
                        mask = m_mid if 1 <= kb <= 3 else (m_first if kb == 0 else m_last)
                        ep = epool.tile([128, 256], F32R, tag="ep")
                        nc.vector.tensor_mul(ep[:, 0:nq], ex[:, 0:nq], mask[:, 0:nq])
                        nc.tensor.matmul(
                            pvp[:, q_lo - 128:q_hi - 128],
                            v_sb[kb][:, h * (DH + 1):(h + 1) * (DH + 1)],
                            ep[:, 0:nq],
                            start=(kb == 0), stop=(kb == 4),
                        )
                    # per-head normalization: row DH holds the denominator
                    rcp0 = rpool.tile([1, CHUNK], F32, tag="rcp0")
                    nc.vector.reciprocal(rcp0[:], pvp[DH:DH + 1, :])
                    rcp = rpool.tile([1, CHUNK], F32R, tag="rcp")
                    nc.vector.tensor_copy(rcp[:], rcp0[:])
                    rb = ps_rb.tile([DH, CHUNK], F32, tag="rb")
                    nc.tensor.matmul(rb[:], ones_col[:], rcp[:], start=True, stop=True)
                    rbs = rpool.tile([DH, CHUNK], F32, tag="rbs")
                    nc.any.tensor_copy(rbs[:], rb[:])
                    nc.vector.tensor_mul(
                        attn[h // 2][po:po + DH, :], pvp[0:DH, :], rbs[:],
                    )

            # ---------------- phase 5: proj ----------------
            with (
                tc.tile_pool(name="outp", bufs=3) as opool,
                tc.tile_pool(name="ps_pj", bufs=3, space="PSUM") as ps_pj,
            ):
                for m4 in range(4):
                    osb = opool.tile([128, C], F32, tag="osb")
                    for nh in range(2):
                        pj = ps_pj.tile([128, 512], F32, tag="pj")
                        for k in range(KT):
                            nc.tensor.matmul(
                                pj[:],
                                attn[k][:, m4 * 128:(m4 + 1) * 128],
                                wp[k][:, nh * 512:(nh + 1) * 512],
                                start=(k == 0), stop=(k == KT - 1),
                            )
                        nc.vector.tensor_add(
                            osb[:, nh * 512:(nh + 1) * 512], pj[:],
                            bp[:, nh * 512:(nh + 1) * 512],
                        )
                    nc.sync.dma_start(out_d[m4 * 128:(m4 + 1) * 128, :], osb[:])

    nc.compile()
    return nc


def _host_tables():
    """RoPE tables ([128, HALO+T] with a zero pad for pre-sequence halo),
    band masks, identity."""
    inv_freq = 1.0 / (ROPE_BASE ** (np.arange(0, DH, 2, dtype=np.float64) / DH))
    t = np.arange(T, dtype=np.float64)
    freqs = t[:, None] * inv_freq[None, :]          # [T, 32]
    cos = np.repeat(np.cos(freqs), 2, axis=1).T     # [64, T]
    sin = np.repeat(np.sin(freqs), 2, axis=1).T
    sin_signed = sin.copy()
    sin_signed[0::2, :] *= -1.0                     # row 2i gets -sin_i
    pad = np.zeros((DH, HALO))
    cosT = np.concatenate([pad, cos], axis=1)       # [64, HALO+T]
    sinT = np.concatenate([pad, sin_signed], axis=1)
    cos2 = np.concatenate([cosT, cosT], axis=0).astype(np.float32)   # [128, .]
    sin2 = np.concatenate([sinT, sinT], axis=0).astype(np.float32)
    scale = 1.0 / np.sqrt(DH)
    tabs = dict(
        cosq=(cos2 * scale).astype(np.float32),
        sinq=(sin2 * scale).astype(np.float32),
        cosk=cos2, sink=sin2,
    )

    p = np.arange(128)[:, None]
    f128 = np.arange(128)[None, :]
    f256 = np.arange(256)[None, :]
    m_first = (f128 < p).astype(np.float32)
    m_mid = ((p <= f256) & (f256 < p + 128)).astype(np.float32)
    m_last = (f128 >= p).astype(np.float32)
    ident = np.eye(128, dtype=np.float32)
    return tabs, m_first, m_mid, m_last, ident


_NC_CACHE = {}


def _prepare_in_maps(x, Wqkv, bqkv, Wproj, bproj):
    x = np.ascontiguousarray(np.asarray(x, dtype=np.float32))
    Wqkv = np.ascontiguousarray(np.asarray(Wqkv, dtype=np.float32))
    bqkv = np.asarray(bqkv, dtype=np.float32)
    Wproj = np.ascontiguousarray(np.asarray(Wproj, dtype=np.float32))
    bproj = np.asarray(bproj, dtype=np.float32)

    tabs, m_first, m_mid, m_last, ident = _host_tables()
    bqk_cols = bqkv[: 2 * C].reshape(NM_QK, 128).T.copy()      # [128, 16]
    bv_rep = np.tile(bqkv[2 * C:], (128, 1)).astype(np.float32)  # [128, C]
    bp_rep = np.tile(bproj, (128, 1)).astype(np.float32)

    zero_mask_first = np.zeros_like(m_first)

    in_maps = []
    for core in range(N_CORES):
        b, ci = divmod(core, 4)
        s = ci * CHUNK
        xc = np.zeros((L, C), dtype=np.float32)
        xc[HALO:] = x[b, s:s + CHUNK]
        if ci > 0:
            xc[:HALO] = x[b, s - HALO:s]
        in_maps.append({
            "x_chunk": xc,
            "wqkv": Wqkv,
            "wproj": Wproj,
            "ident": ident,
            "cosq": tabs["cosq"][:, s:s + L].copy(),
            "sinq": tabs["sinq"][:, s:s + L].copy(),
            "cosk": tabs["cosk"][:, s:s + L].copy(),
            "sink": tabs["sink"][:, s:s + L].copy(),
            "mask_first": m_first if ci > 0 else zero_mask_first,
            "mask_mid": m_mid,
            "mask_last": m_last,
            "bqk_cols": bqk_cols,
            "bv_rep": bv_rep,
            "bp_rep": bp_rep,
            "ones_col": np.ones((1, DH), dtype=np.float32),
            "ones_v": np.ones((128, H), dtype=np.float32),
        })
    return in_maps


def kernel(x, Wqkv, bqkv, Wproj, bproj, _trace=False):
    if "nc" not in _NC_CACHE:
        _NC_CACHE["nc"] = build_nc()
    nc = _NC_CACHE["nc"]

    in_maps = _prepare_in_maps(x, Wqkv, bqkv, Wproj, bproj)

    res = run_bass_kernel_spmd(
        nc, in_maps, core_ids=list(range(N_CORES)), trace=_trace,
    )

    out = np.empty((B, T, C), dtype=np.float32)
    for core in range(N_CORES):
        b, ci = divmod(core, 4)
        s = ci * CHUNK
        out[b, s:s + CHUNK] = res.results[core]["out_chunk"]

    if _trace:
        return out, res
    return out
